# revision 40
# baseline (speedup 1.0000x reference)
# Bass/Tile TRN2 kernel for nn_Conv1D_style: out = ((x * (cluster@style_L)) @ weight) * (cluster@style_R)
#
# Sharding: data-parallel over the batch dim. Each of the 8 cores gets a
# 1024-row slice of x/cluster and a full (replicated) weight/style_L/style_R.
#
# Per-core plan (M=1024 batch, K=4096 din, N=4096 dout):
#   aT[k] = xT[k] * (style_L[:, kslice].T @ clusterT)
#   y[m,n] = sum_k aT[k][:, mslice].T @ W[k, nslice]
#   out[m,n] = y[m,n] * (clusterT[:, mslice].T @ style_R[:, nslice])
#
# Precision split: the first NF8=8 k-tiles (1024 of 4096 din) run as fp8
# e4m3 DoubleRow matmuls — each DoubleRow pass contracts a PAIR of k-tiles
# (256-deep) in the cycles of one bf16 512-col stream, halving PE time for
# that fraction. The remaining 24 k-tiles stay bf16. Measured rel error of
# this split vs the fp32 reference is ~1.8e-2 (gate 2e-2); per-k errors are
# iid so the error scales as sqrt(NF8/32) of full-fp8's 3.56e-2. A values
# peak at ~188, safely under the TRN e4m3 ±240 ceiling.
#
# The aT production is fused with the first n-block's accumulation. The
# K=64 style matmuls are row-packed two at a time via tile_position. DMA:
# xT/consts on the Activation queue (sL split so the first pl matmul isn't
# gated on the full 1 MiB; xg granules land in halves), W (w8 then wbf,
# first-n in granules) on Sync, sR + outputs on the GpSimd queue.

import numpy as np
import ml_dtypes

B, DIN, DOUT, NCL = 8192, 4096, 4096, 64
NCORES = 8
MB = B // NCORES          # batch rows per core
P = 128
NT = 512                  # n tile (dout cols per matmul)
KT = DIN // P             # 32 k tiles
MT = MB // P              # 8 m tiles
NTS = DOUT // NT          # 8 n tiles
FUSED = 4                 # m tiles of n=0 accumulated during the aT prologue
XG = 4                    # k tiles per xT DMA granule
NF8 = 8                   # leading k-tiles computed in fp8 e4m3 (DoubleRow)
NP8 = NF8 // 2            # DoubleRow k-tile pairs
NBF = KT - NF8            # trailing k-tiles in bf16

_CACHE = {}
LAST = {}                 # exposes the most recent BassKernelResults for test harnesses


def _build_program():
    import concourse.bacc as bacc
    import concourse.mybir as mybir
    import concourse.tile as tile

    bf16 = mybir.dt.bfloat16
    fp8 = mybir.dt.float8e4
    f32 = mybir.dt.float32
    DR = mybir.MatmulPerfMode.DoubleRow

    nc = bacc.Bacc(None, target_bir_lowering=False, debug=False)

    # xT: [granule, partition, k-in-granule, batch]
    # w8: [n, partition, k-pair, plane, nt] fp8 (logical k = (2t+i)*128+p)
    # wbf: [n, partition, k-8, nt] bf16 (k-tiles 8..31)
    # cluster/styles arrive duplicated: rows 64-127 = rows 0-63 (row packing).
    xT_d = nc.declare_dram_parameter("xT", [KT // XG, P, XG, MB], bf16, isOutput=False)
    clT_d = nc.declare_dram_parameter("clusterT", [P, MB], bf16, isOutput=False)
    w8_d = nc.declare_dram_parameter("w8", [NTS, P, NP8, 2, NT], fp8, isOutput=False)
    w_d = nc.declare_dram_parameter("weight", [NTS, P, NBF, NT], bf16, isOutput=False)
    sL_d = nc.declare_dram_parameter("style_L", [P, DIN], bf16, isOutput=False)
    sR_d = nc.declare_dram_parameter("style_R", [P, DOUT], bf16, isOutput=False)
    out_d = nc.declare_dram_parameter("out", [MB, DOUT], f32, isOutput=True)

    H = NCL  # 64: row-pack halves

    with tile.TileContext(nc) as tc:
        with (
            tc.tile_pool(name="const", bufs=1) as const_pool,
            tc.tile_pool(name="at8p", bufs=1) as at8_pool,
            tc.tile_pool(name="atp", bufs=1) as at_pool,
            tc.tile_pool(name="w8p", bufs=2) as w8_pool,
            tc.tile_pool(name="wp", bufs=2) as w_pool,
            tc.tile_pool(name="xp", bufs=4) as x_pool,
            tc.tile_pool(name="evp", bufs=3) as ev_pool,
            # PSUM budget (8 banks): py 4 x [128,512] (tmpR psum + y
            # accumulators) + pl 2 x [128,1024] (2 banks each) = 8.
            tc.tile_pool(name="pyp", bufs=4, space="PSUM") as py_pool,
            tc.tile_pool(name="plp", bufs=2, space="PSUM") as pl_pool,
        ):
            # ---- constants. Every DMA queue needs ~19us before its first
            # ~512 KiB lands (engine preamble + cold HWDGE ramp), so spread
            # the critical first tiles one-per-queue: xg0 leads Activation
            # (via issue_xg below), clT+sL head the Sync queue ahead of W,
            # and the sL tail leads GpSimd ahead of the odd xg granules.
            clT = const_pool.tile([P, MB], bf16, name="clT")
            sL = const_pool.tile([P, DIN], bf16, name="sL")
            sR = const_pool.tile([P, DOUT], bf16, name="sR")
            nc.sync.dma_start(clT[:], clT_d[:])
            nc.sync.dma_start(sL[:, 0:NF8 * P], sL_d[:, 0:NF8 * P])
            nc.gpsimd.dma_start(sL[:, NF8 * P:], sL_d[:, NF8 * P:])

            def tmpr_pair(n, m, psum_src="py"):
                """Row-packed pair: tmpR tiles for (m, m+1) at n, staged to SBUF.

                psum_src="pl" borrows a pl-pool tile (two banks) instead of two
                py slots — required in the fused prologue where all four py
                slots are held by the open accumulators (a py allocation there
                would deadlock against its own epilogue).
                """
                if psum_src == "pl":
                    prp = pl_pool.tile([P, MB], f32, name=f"prf{n}_{m}", tag="pl")
                    pra, prb = prp[:, 0:NT], prp[:, NT:MB]
                else:
                    pra = py_pool.tile([P, NT], f32, name=f"pr{n}_{m}", tag="py")
                    prb = py_pool.tile([P, NT], f32, name=f"pr{n}_{m + 1}", tag="py")
                # K=128 over the duplicated halves (styles pre-halved on the
                # host): identical math to the K=64 contraction, but no PE
                # tile-config switch mid-stream
                nc.tensor.matmul(
                    pra[:],
                    clT[:, m * P:(m + 1) * P],
                    sR[:, n * NT:(n + 1) * NT],
                    start=True, stop=True,
                )
                nc.tensor.matmul(
                    prb[:],
                    clT[:, (m + 1) * P:(m + 2) * P],
                    sR[:, n * NT:(n + 1) * NT],
                    start=True, stop=True,
                )
                # tr staged as bf16: halves the DVE copy/read traffic during
                # the production window; adds ~0.1% output error (quadrature
                # vs the 1.8e-2 fp8 term: negligible). Kept off the scalar
                # engine so its stream keeps issuing xT granule DMAs promptly.
                tra = ev_pool.tile([P, NT], bf16, name=f"tr{n}_{m}", tag="tr", bufs=6)
                trb = ev_pool.tile([P, NT], bf16, name=f"tr{n}_{m + 1}", tag="tr", bufs=6)
                nc.vector.tensor_copy(out=tra[:], in_=pra[:])
                nc.vector.tensor_copy(out=trb[:], in_=prb[:])
                return tra, trb

            def epilogue(n, m, py, tr):
                ot = ev_pool.tile([P, NT], f32, name=f"ot{n}_{m}", tag="ot")
                nc.vector.tensor_mul(out=ot[:], in0=py[:], in1=tr[:])
                nc.sync.dma_start(
                    out_d[m * P:(m + 1) * P, n * NT:(n + 1) * NT], ot[:]
                )

            # ---- W for n=0: fp8 pairs first (they open every accumulation
            # group), then bf16 in granules so the fused prologue's first
            # bf16 MM isn't gated on the whole 3 MiB ----
            w80 = w8_pool.tile([P, NP8, 2, NT], fp8, name="w80", tag="w8")
            nc.sync.dma_start(w80[:], w8_d[0])
            w0 = w_pool.tile([P, NBF, NT], bf16, name="w0", tag="wbig")
            nc.sync.dma_start(w0[:, 0:8, :], w_d[0, :, 0:8, :])
            nc.sync.dma_start(sR[:], sR_d[:])
            for j in range(1, NBF // 8):
                nc.sync.dma_start(
                    w0[:, j * 8:(j + 1) * 8, :],
                    w_d[0, :, j * 8:(j + 1) * 8, :],
                )

            # ---- fused prologue: aT production + n0/m0..3 k-outer accumulation ----
            py_f = [
                py_pool.tile([P, NT], f32, name=f"py0_{m}", tag="py")
                for m in range(FUSED)
            ]
            at8_tiles = [
                at8_pool.tile([P, 2, MB], fp8, name=f"at8_{t}", tag=f"at8_{t}")
                for t in range(NP8)
            ]
            at_tiles = []
            tr_f = []
            # alternate the xT stream between the Activation and Vector HWDGE
            # queues (one queue, ~200 GB/s effective, cannot feed the fused
            # prologue's aT production) and issue two granules ahead so the
            # odd-granule dma_starts aren't stuck behind the vector engine's
            # tensor_muls in its instruction stream.
            xg_tiles = {}

            def issue_xg(g):
                xg = x_pool.tile([P, XG, MB], bf16, name=f"xg{g}", tag="xg")
                xq = nc.scalar if g % 2 == 0 else nc.gpsimd
                # half-granule (512 KiB) transfers: the HWDGE ring keeps only
                # ~2 transfers in flight per queue, so finer slicing starves
                # the lookahead and coarser slicing delays the first consumer
                xq.dma_start(xg[:, 0:XG // 2, :], xT_d[g, :, 0:XG // 2, :])
                xq.dma_start(xg[:, XG // 2:, :], xT_d[g, :, XG // 2:, :])
                xg_tiles[g] = xg

            issue_xg(0)
            issue_xg(1)
            issue_xg(2)
            for g in range(KT // XG):
                if g + 3 < KT // XG:
                    issue_xg(g + 3)
                xg = xg_tiles.pop(g)
                for j in range(XG):
                    k = g * XG + j
                    # tmpLT: two per-PSUM-bank halves (a single matmul output
                    # may not cross a bank boundary), each K=128 over the
                    # duplicated rows with pre-halved sL — no tile-config
                    # switches and both halves share one stationary load
                    pl = pl_pool.tile([P, MB], f32, name=f"pl{k}", tag="pl")
                    nc.tensor.matmul(
                        pl[:, 0:NT],
                        sL[:, k * P:(k + 1) * P],
                        clT[:, 0:NT],
                        start=True, stop=True,
                    )
                    nc.tensor.matmul(
                        pl[:, NT:MB],
                        sL[:, k * P:(k + 1) * P],
                        clT[:, NT:MB],
                        start=True, stop=True,
                    )
                    if k < NF8:
                        t, i = k // 2, k % 2
                        nc.vector.tensor_mul(
                            out=at8_tiles[t][:, i, :], in0=xg[:, j, :], in1=pl[:]
                        )
                        if i == 1:
                            for m in range(FUSED):
                                nc.tensor.matmul(
                                    py_f[m][:],
                                    at8_tiles[t][:, :, m * P:(m + 1) * P],
                                    w80[:, t, :, :],
                                    start=(t == 0), stop=False, perf_mode=DR,
                                )
                    else:
                        at_k = at_pool.tile([P, MB], bf16, name=f"at{k}", tag=f"at{k}")
                        nc.vector.tensor_mul(out=at_k[:], in0=xg[:, j, :], in1=pl[:])
                        at_tiles.append(at_k)
                        for m in range(FUSED):
                            nc.tensor.matmul(
                                py_f[m][:],
                                at_k[:, m * P:(m + 1) * P],
                                w0[:, k - NF8, :],
                                start=False, stop=(k == KT - 1),
                            )
                if g == 1:
                    # tmpR for the fused m tiles; placed here (PE has slack in
                    # the prologue) so it doesn't gate the kernel start on sR
                    tr_f += tmpr_pair(0, 0, psum_src="pl")
                elif g == 2:
                    tr_f += tmpr_pair(0, 2, psum_src="pl")
            for m in range(FUSED):
                epilogue(0, m, py_f[m], tr_f[m])

            # ---- standard m-pair body: fp8 DoubleRow pairs open the group,
            # bf16 k-tiles close it; the packed tmpR pair is injected
            # mid-group (the deep MM pipeline hides its LDWEIGHTS) ----
            def body_pair(n, m, w8n, wn):
                # both m-tiles' DoubleRow matmuls run as ONE fp8 stretch:
                # each fp8<->bf16 perf-mode seam costs a ~200-400ns pipeline
                # bubble, so pay it twice per pair instead of four times
                pys = [
                    py_pool.tile([P, NT], f32, name=f"py{n}_{mm}", tag="py")
                    for mm in (m, m + 1)
                ]
                for i, mm in enumerate((m, m + 1)):
                    for t in range(NP8):
                        nc.tensor.matmul(
                            pys[i][:],
                            at8_tiles[t][:, :, mm * P:(mm + 1) * P],
                            w8n[:, t, :, :],
                            start=(t == 0), stop=False, perf_mode=DR,
                        )
                tra = trb = None
                for i, mm in enumerate((m, m + 1)):
                    for kb in range(NBF):
                        nc.tensor.matmul(
                            pys[i][:],
                            at_tiles[kb][:, mm * P:(mm + 1) * P],
                            wn[:, kb, :],
                            start=False, stop=(kb == NBF - 1),
                        )
                        if mm == m and kb == 8:
                            tra, trb = tmpr_pair(n, m)
                    epilogue(n, mm, pys[i], tra if mm == m else trb)

            # rest of n=0
            for m in range(FUSED, MT, 2):
                body_pair(0, m, w80, w0)
            # n = 1..7
            for n in range(1, NTS):
                w8n = w8_pool.tile([P, NP8, 2, NT], fp8, name=f"w8{n}", tag="w8")
                nc.sync.dma_start(w8n[:], w8_d[n])
                wn = w_pool.tile([P, NBF, NT], bf16, name=f"w{n}", tag="wbig")
                nc.sync.dma_start(wn[:], w_d[n])
                for m in range(0, MT, 2):
                    body_pair(n, m, w8n, wn)

    nc.finalize()
    return nc


def _get_program():
    if "nc" not in _CACHE:
        _CACHE["nc"] = _build_program()
    return _CACHE["nc"]


def _prep_inputs(x, cluster, weight, style_L, style_R):
    bf16 = ml_dtypes.bfloat16
    e4m3 = ml_dtypes.float8_e4m3

    wf = np.asarray(weight, dtype=np.float32)
    # fp8 k-range: rows 0..NF8*128, logical k = (2t+i)*128+p -> [n,p,t,i,nt]
    w8_r = np.ascontiguousarray(
        wf[:NF8 * P].astype(e4m3).reshape(NP8, 2, P, NTS, NT).transpose(3, 2, 0, 1, 4)
    )
    # bf16 k-range: [din, dout] -> [n, p, k, nt] partition-major
    w_r = np.ascontiguousarray(
        wf[NF8 * P:].astype(bf16).reshape(NBF, P, NTS, NT).transpose(2, 1, 0, 3)
    )
    # styles duplicated across both 64-row halves and pre-halved: K=128
    # matmuls over the duplicated rows then equal the K=64 contraction
    # (x0.5 is exact in bf16)
    sL1 = (np.asarray(style_L, dtype=np.float32) * 0.5).astype(bf16)
    sR1 = (np.asarray(style_R, dtype=np.float32) * 0.5).astype(bf16)
    sL = np.ascontiguousarray(np.vstack([sL1, sL1]))
    sR = np.ascontiguousarray(np.vstack([sR1, sR1]))

    in_maps = []
    for c in range(NCORES):
        xs = np.asarray(x[c * MB:(c + 1) * MB], dtype=np.float32)
        xT = np.ascontiguousarray(xs.T).astype(bf16)          # [DIN, MB]
        # [din, mb] -> [granule, p, k-in-granule, mb]
        xT_r = np.ascontiguousarray(
            xT.reshape(KT // XG, XG, P, MB).transpose(0, 2, 1, 3)
        )
        clT1 = np.ascontiguousarray(
            np.asarray(cluster[c * MB:(c + 1) * MB], dtype=np.float32).T
        ).astype(bf16)
        clT = np.ascontiguousarray(np.vstack([clT1, clT1]))
        in_maps.append(
            {"xT": xT_r, "clusterT": clT, "w8": w8_r, "weight": w_r,
             "style_L": sL, "style_R": sR}
        )
    return in_maps


def kernel(x, cluster, weight, style_L, style_R):
    import os

    # The NTFF trace path needs an antenv hook this container lacks; never
    # let a stray BASS_TRACE env take the run down that path.
    os.environ.setdefault("BASS_NEVER_TRACE", "1")
    from concourse.bass_utils import run_bass_kernel_spmd

    nc = _get_program()
    in_maps = _prep_inputs(x, cluster, weight, style_L, style_R)

    res = run_bass_kernel_spmd(nc, in_maps, list(range(NCORES)))
    LAST["results"] = res
    LAST["in_maps"] = in_maps
    out = np.concatenate(
        [np.asarray(res.results[c]["out"], dtype=np.float32) for c in range(NCORES)],
        axis=0,
    )
    return out
